# revision 18
# baseline (speedup 1.0000x reference)
"""Bass/Tile kernel for nn_Attn_40424232189956 on 8 trn2 NeuronCores.

GQA attention block: q/k/v proj + rmsnorm + rope + causal attention + out proj.
B=2, T=2048, D=2048, NH=16, NKV=4, HD=128.

Sharding: 4 q-heads x 1 batch per core (core c: batch c//4, q heads
4*(c%4)..4*(c%4)+3, kv head c%4). Each (batch, kv head) pair is computed by
exactly one core -> no duplicated kv projection work. Each core emits a full
[T, D] partial of the output projection for its batch; host sums the 4
partials per batch.

Per-core kernel layout:
- Projections feat-major: psum [feat 128, tok 512], lhsT = W^T k-tiles,
  rhs = x^T k-tiles (x transposed + cast to bf16 on host). One batched DMA
  per 512-token chunk loads all 16 k-tiles.
- RMSNorm via ones-matmul partition reduction (value 1/(128*s_h^2) folds the
  qg gain and softmax 1/sqrt(HD) into the norm factor), sqrt bias eps/s_h^2.
- Rope in hd-major reading q halves straight from PSUM.
- Attention with TRANSPOSED scores sT[kt, qt]: softmax denominator via
  ones-column matmul (partition reduction on PE), p used directly as rhs of
  the pv matmul. exp() without max-subtraction (scores bounded by sqrt(HD)
  after rmsnorm). Heads processed in 2 passes of 2 (psum budget); the j-loop
  is software-pipelined: scores for step j issue before the sms/pv matmuls
  of step j-1, so the PE never waits on the exp() round trip.
- Causal masking: additive -30000 masks for the 4 diagonal block phases.
- Output written bf16 [T, D]; host sums partials in f32.
"""

import numpy as np

B, T, D = 2, 2048, 2048
NH, NKV = 16, 4
HD = 128
NCORES = 8
HPC = 4               # q heads per core
NKT = D // 128        # 16 contraction tiles for projections
CHUNK = 512
NCH = T // CHUNK      # 4 chunks
EPS = float(np.finfo(np.float32).eps)
MASK_NEG = -30000.0


def _rope_tables():
    # Matches reference.rotary_tables for T=2048 > tsl=1024 (NTK branch).
    hd = np.float32(HD)
    ar = (np.arange(0, HD, 2, dtype=np.float32) / hd).astype(np.float32)
    expo = np.power(np.float32(HD / (HD - 2.0)), ar, dtype=np.float32)
    inv = (np.float32(1.0)
           / (np.float32(10000.0)
              * np.power(np.float32(T / 1024.0), expo, dtype=np.float32)))
    f = np.outer(np.arange(T, dtype=np.float32), inv.astype(np.float32))
    return (np.cos(f).astype(np.float32).T.copy(),
            np.sin(f).astype(np.float32).T.copy())  # [64, T] hd-major


def _build_program():
    import concourse.bass as bass
    import concourse.mybir as mybir
    import concourse.tile as tile
    from concourse import bacc
    from concourse.masks import make_identity

    f32 = mybir.dt.float32
    f32r = mybir.dt.float32r
    bf16 = mybir.dt.bfloat16
    nc = bacc.Bacc("TRN2", target_bir_lowering=False)

    # lhsT (stationary) tensors are float32r: 4-byte weights self-load inside
    # the matmul, so tile_legalize emits no separate Ldweights instruction
    # (saves ~100ns of PE sequencer time per matmul). rhs (moving) tensors
    # are bf16: the PE row rate is keyed on the moving dtype.
    xT = nc.dram_tensor("xT", [D, T], f32r, kind="ExternalInput")
    qwT = nc.dram_tensor("qwT", [D, HPC * HD], f32r, kind="ExternalInput")
    kwT = nc.dram_tensor("kwT", [D, HD], f32r, kind="ExternalInput")
    vwT = nc.dram_tensor("vwT", [D, HD], f32r, kind="ExternalInput")
    owT = nc.dram_tensor("owT", [HPC * HD, D], bf16, kind="ExternalInput")
    csd = nc.dram_tensor("csd", [128, T], f32, kind="ExternalInput")
    csd2 = nc.dram_tensor("csd2", [128, T], f32, kind="ExternalInput")
    maskd = nc.dram_tensor("maskd", [128, 4, CHUNK], f32r, kind="ExternalInput")
    identd = nc.dram_tensor("identd", [128, 128], f32r, kind="ExternalInput")
    normod = nc.dram_tensor("normod", [128, HPC + 1, 128], f32r,
                            kind="ExternalInput")
    normbd = nc.dram_tensor("normbd", [128, HPC + 1], f32, kind="ExternalInput")
    onesd = nc.dram_tensor("onesd", [128, 1], f32r, kind="ExternalInput")
    outd = nc.dram_tensor("o", [T, D], bf16, kind="ExternalOutput")

    with tile.TileContext(nc) as tc:
        with (
            tc.tile_pool(name="wpool", bufs=1) as wpool,
            tc.tile_pool(name="xpool", bufs=2) as xpool,
            tc.tile_pool(name="big", bufs=1) as big,
            tc.tile_pool(name="ybp", bufs=2) as ybp,
            tc.tile_pool(name="ntmp", bufs=2) as ntmp,
            tc.tile_pool(name="ntmp1", bufs=2) as ntmp1,
            tc.tile_pool(name="atmp", bufs=2) as atmp,
            tc.tile_pool(name="ppool", bufs=4) as ppool,
            tc.tile_pool(name="opool", bufs=2) as opool,
            tc.tile_pool(name="ps", bufs=6, space="PSUM") as ps,
            tc.tile_pool(name="psv", bufs=2, space="PSUM") as psv,
        ):
            # ---- resident weights / tables ----
            # First x half-chunk and the first quarter of qw + all of kw/vw
            # are loaded before the bulkier tables so the first projection
            # matmuls start as early as possible.
            NQ = NKT // 4
            xc0 = xpool.tile([128, NQ, CHUNK], f32r, tag="xc",
                             name="xc_0_0")
            xr = xT.rearrange("(ko p) m -> p ko m", p=128)
            nc.sync.dma_start(xc0[:], xr[:, 0:NQ, 0:CHUNK])
            qw_s = wpool.tile([128, NKT, HPC * HD], f32r)
            qwr = qwT.rearrange("(ko p) m -> p ko m", p=128)
            nc.sync.dma_start(qw_s[:, 0:4, :], qwr[:, 0:4, :])
            kw_s = wpool.tile([128, NKT, HD], f32r)
            nc.sync.dma_start(kw_s[:], kwT.rearrange("(ko p) m -> p ko m", p=128))
            vw_s = wpool.tile([128, NKT, HD], f32r)
            nc.sync.dma_start(vw_s[:], vwT.rearrange("(ko p) m -> p ko m", p=128))
            normo_s = wpool.tile([128, HPC + 1, 128], f32r)
            nc.sync.dma_start(normo_s[:], normod[:])
            normb_s = wpool.tile([128, HPC + 1], f32)
            nc.sync.dma_start(normb_s[:], normbd[:])
            for qq in range(1, 4):
                nc.sync.dma_start(qw_s[:, 4 * qq:4 * qq + 4, :],
                                  qwr[:, 4 * qq:4 * qq + 4, :])
            cs_s = wpool.tile([128, T], f32)  # rows 0:64 cos, 64:128 sin
            nc.sync.dma_start(cs_s[:], csd[:])
            cs2_s = wpool.tile([128, T], f32)  # rows 0:64 sin, 64:128 cos
            nc.sync.dma_start(cs2_s[:], csd2[:])
            ow_s = wpool.tile([128, HPC, D], bf16)
            nc.sync.dma_start(ow_s[:], owT.rearrange("(h p) n -> p h n", p=128))
            mask_s = wpool.tile([128, 4, CHUNK], f32r)
            nc.sync.dma_start(mask_s[:], maskd[:])
            identneg = wpool.tile([128, 128], f32r)
            nc.sync.dma_start(identneg[:], identd[:])
            ones_col = wpool.tile([128, 1], f32r)
            nc.sync.dma_start(ones_col[:], onesd[:])
            ident = wpool.tile([128, 128], f32)
            make_identity(nc, ident[:])

            qT = big.tile([128, HPC, T], bf16, tag="qT", name="qT")
            kT = big.tile([128, T], bf16, tag="kT", name="kT")
            vtok = big.tile([128, T], f32r, tag="vtok", name="vtok")

            sq_ = mybir.ActivationFunctionType.Square
            sqrt_ = mybir.ActivationFunctionType.Sqrt
            exp_ = mybir.ActivationFunctionType.Exp

            def norm_rope(pt, ni, dst, pos0):
                """pt: psum [128 feat, 512 tok]; ni: 0..3 q-head, 4 k;
                dst: sbuf [128, 512] slice; pos0: seq position of col 0."""
                sq = ntmp.tile([128, CHUNK], f32r, tag="sq")
                nc.scalar.activation(out=sq[:], in_=pt[:], func=sq_)
                # full-width copy: every rope operand then lives in SBUF, so
                # the projection PSUM bank frees after just two ACT reads
                qsb = ntmp.tile([128, CHUNK], f32, tag="qsb")
                nc.scalar.copy(out=qsb[:], in_=pt[:])
                nb = psv.tile([128, CHUNK], f32, tag="aux", name=f"nb_{ni}_{pos0}")
                nc.tensor.matmul(nb[:], normo_s[:, ni, :], sq[:],
                                 start=True, stop=True)
                rs = ntmp1.tile([64, CHUNK], f32, tag="rs")
                nc.scalar.activation(out=rs[:], in_=nb[0:64, :], func=sqrt_,
                                     bias=normb_s[0:64, ni:ni + 1], scale=1.0)
                rfac = ntmp1.tile([64, CHUNK], f32, tag="rf")
                nc.vector.reciprocal(rfac[:], rs[:])
                cs = cs_s[0:64, pos0:pos0 + CHUNK]       # cos @ base 0
                sn = cs_s[64:128, pos0:pos0 + CHUNK]     # sin @ base 64
                sn0 = cs2_s[0:64, pos0:pos0 + CHUNK]     # sin @ base 0
                cs64 = cs2_s[64:128, pos0:pos0 + CHUNK]  # cos @ base 64
                # three multiplies on the idle GPSIMD engine (sbuf-only
                # operands; each operand pair shares a partition window)
                t1 = ntmp1.tile([64, CHUNK], f32, tag="ta")
                t2 = ntmp1.tile([64, CHUNK], f32, tag="tb")
                nc.gpsimd.tensor_mul(t1[:], qsb[0:64, :], cs)
                nc.gpsimd.tensor_mul(t2[:], qsb[64:128, :], sn)
                nc.vector.tensor_add(t1[:], t1[:], t2[:])
                nc.vector.tensor_mul(dst[0:64, :], t1[:], rfac[:])
                t3 = ntmp1.tile([64, CHUNK], f32, tag="tb")
                t4 = ntmp1.tile([64, CHUNK], f32, tag="ta")
                nc.gpsimd.tensor_mul(t3[:], qsb[0:64, :], sn0)
                nc.vector.tensor_mul(t4[:], qsb[64:128, :], cs64)
                nc.vector.tensor_sub(t4[:], t4[:], t3[:])
                nc.vector.tensor_mul(dst[64:128, :], t4[:], rfac[:])

            def proj_chunk(ci):
                pos0 = ci * CHUNK
                if ci == 0:
                    xh = [xc0]
                else:
                    xh = [xpool.tile([128, NQ, CHUNK], f32r, tag="xc",
                                     name=f"xc_{ci}_0")]
                    nc.sync.dma_start(
                        xh[0][:], xr[:, 0:NQ, pos0:pos0 + CHUNK])
                for qq in range(1, 4):
                    xh.append(xpool.tile([128, NQ, CHUNK], f32r, tag="xc",
                                         name=f"xc_{ci}_{qq}"))
                    nc.sync.dma_start(
                        xh[qq][:],
                        xr[:, NQ * qq:NQ * (qq + 1), pos0:pos0 + CHUNK])
                pq = [ps.tile([128, CHUNK], f32, tag="b512",
                              name=f"pq{h}_{ci}") for h in range(HPC)]
                pk = ps.tile([128, CHUNK], f32, tag="b512", name=f"pk_{ci}")
                pv = ps.tile([128, CHUNK], f32, tag="b512", name=f"pv_{ci}")
                for ko in range(NKT):
                    rhs = xh[ko // NQ][:, ko % NQ, :]
                    st = (ko == 0)
                    sp = (ko == NKT - 1)
                    for h in range(HPC):
                        nc.tensor.matmul(pq[h][:],
                                         qw_s[:, ko, h * 128:(h + 1) * 128],
                                         rhs, start=st, stop=sp)
                    nc.tensor.matmul(pk[:], kw_s[:, ko, :], rhs,
                                     start=st, stop=sp)
                    nc.tensor.matmul(pv[:], vw_s[:, ko, :], rhs,
                                     start=st, stop=sp)
                # v first: psum [hd, tok] -> sbuf, PE-transpose to token-major
                vtmp = atmp.tile([128, CHUNK], f32, tag="vtmp",
                                 name=f"vtmp_{ci}")
                nc.scalar.copy(out=vtmp[:], in_=pv[:])
                for tb in range(4):
                    vps = psv.tile([128, CHUNK], f32, tag="aux",
                                   name=f"vps_{ci}_{tb}")[:, 0:128]
                    nc.tensor.transpose(
                        vps, vtmp[:, tb * 128:(tb + 1) * 128], ident[:])
                    dst0 = pos0 + tb * 128
                    nc.scalar.copy(out=vtok[:, dst0:dst0 + 128], in_=vps)
                # K norm/rope first: the next attention group's scores need it
                norm_rope(pk, HPC, kT[:, pos0:pos0 + CHUNK], pos0)
                for h in range(HPC):
                    norm_rope(pq[h], h, qT[:, h, pos0:pos0 + CHUNK], pos0)

            def attn_pass(g, hh):
                """Attention for query group g, heads hh (pair). Writes
                normalized per-head outputs into ybg[:, h, :]."""
                q0 = g * CHUNK
                kg = 4 * (g + 1)
                yts, sms = {}, {}
                for h in hh:
                    yts[h] = ps.tile([128, CHUNK], f32, tag="b512",
                                     name=f"yt_{g}_{h}")
                    sms[h] = psv.tile([128, CHUNK], f32, tag="aux",
                                      name=f"sm_{g}_{h}")[0:1, :]
                pend = None  # (j, {h: pj}) awaiting sms/pv issue
                for j in range(kg):
                    k0 = j * 128
                    st = {}
                    diag = j >= 4 * g
                    for h in hh:  # both scores share the kT lhsT
                        st[h] = ps.tile([128, CHUNK], f32, tag="b512",
                                        name=f"st_{g}_{h}_{j}")
                        nc.tensor.matmul(st[h][:], kT[:, k0:k0 + 128],
                                         qT[:, h, q0:q0 + CHUNK],
                                         start=True, stop=not diag)
                        if diag:
                            # causal mask on the PE: -30000*I @ mask01
                            # accumulates the additive mask into the score
                            # bank, keeping the DVE out of the exp chain
                            nc.tensor.matmul(st[h][:], identneg[:],
                                             mask_s[:, j - 4 * g, :],
                                             start=False, stop=True,
                                             skip_group_check=True)
                    if pend is not None:
                        pj_, j_ = pend
                        for h in hh:
                            nc.tensor.matmul(sms[h], ones_col[:], pj_[h][:],
                                             start=(j_ == 0), stop=False,
                                             skip_group_check=True)
                        for h in hh:
                            nc.tensor.matmul(yts[h][:],
                                             vtok[:, j_ * 128:j_ * 128 + 128],
                                             pj_[h][:],
                                             start=(j_ == 0), stop=False,
                                             skip_group_check=True)
                    pjs = {}
                    for h in hh:
                        pj = ppool.tile([128, CHUNK], f32r, tag="pj",
                                        name=f"pj_{g}_{h}_{j}")
                        nc.scalar.activation(out=pj[:], in_=st[h][:],
                                             func=exp_)
                        pjs[h] = pj
                    pend = (pjs, j)
                pj_, j_ = pend
                for h in hh:
                    nc.tensor.matmul(sms[h], ones_col[:], pj_[h][:],
                                     start=(j_ == 0), stop=True,
                                     skip_group_check=True)
                for h in hh:
                    nc.tensor.matmul(yts[h][:],
                                     vtok[:, j_ * 128:j_ * 128 + 128],
                                     pj_[h][:], start=(j_ == 0), stop=True,
                                     skip_group_check=True)
                for h in hh:
                    rrow = atmp.tile([1, CHUNK], f32, tag="rrow",
                                     name=f"rr_{g}_{h}")
                    nc.vector.reciprocal(rrow[:], sms[h])
                    rb = atmp.tile([128, CHUNK], f32, tag="rb",
                                   name=f"rb_{g}_{h}")
                    nc.gpsimd.partition_broadcast(rb[:], rrow[:])
                    nc.vector.tensor_mul(ybg_cur[0][:, h, :], yts[h][:], rb[:])

            def oproj_group(g):
                q0 = g * CHUNK
                ybg = ybg_cur[0]
                for tb in range(4):
                    row0 = q0 + tb * 128
                    ops = [ps.tile([128, CHUNK], f32, tag="b512",
                                   name=f"op_{g}_{tb}_{oc}")
                           for oc in range(4)]
                    for h in range(HPC):
                        lhsT = ybg[:, h, tb * 128:(tb + 1) * 128]
                        for oc in range(4):
                            nc.tensor.matmul(
                                ops[oc][:], lhsT,
                                ow_s[:, h, oc * 512:(oc + 1) * 512],
                                start=(h == 0), stop=(h == HPC - 1),
                                skip_group_check=True)
                    orow = opool.tile([128, D], bf16, tag="orow",
                                      name=f"or_{g}_{tb}")
                    for oc in range(4):
                        dst = orow[:, oc * 512:(oc + 1) * 512]
                        if oc % 2 == 0:
                            nc.vector.tensor_copy(out=dst, in_=ops[oc][:])
                        else:
                            nc.scalar.copy(out=dst, in_=ops[oc][:])
                    nc.sync.dma_start(outd[row0:row0 + 128, :], orow[:])

            ybg_cur = [None]
            # Projection of chunk c+2 is emitted before attention group c+1:
            # its norm/rope chain (DVE+GPSIMD) resolves while the PE runs the
            # attention matmuls, so group c+1 never waits on fresh q/k tiles.
            proj_chunk(0)
            proj_chunk(1)
            for c in range(NCH):
                ybg_cur[0] = ybp.tile([128, HPC, CHUNK], bf16, tag="ybg",
                                      name=f"ybg_{c}")
                attn_pass(c, (0, 1))
                attn_pass(c, (2, 3))
                oproj_group(c)
                if c + 2 < NCH:
                    proj_chunk(c + 2)

    nc.compile()
    return nc


_CACHED = {}
LAST_EXEC_NS = None


def _run(nc, in_maps, **kwargs):
    from concourse.bass_utils import run_bass_kernel_spmd
    return run_bass_kernel_spmd(nc, in_maps, core_ids=list(range(NCORES)),
                                **kwargs)


def _make_in_maps(x, qw, kw, vw, ow, qg):
    import ml_dtypes
    bf = ml_dtypes.bfloat16
    cosT, sinT = _rope_tables()
    cossin = np.concatenate([cosT, sinT], axis=0)   # [128, T] cos||sin
    sincos = np.concatenate([sinT, cosT], axis=0)   # [128, T] sin||cos

    ktl = np.arange(128, dtype=np.int64)[:, None]
    qtl = np.arange(CHUNK, dtype=np.int64)[None, :]
    mask = np.zeros((128, 4, CHUNK), np.float32)
    for r in range(4):
        mask[:, r, :] = np.where(qtl >= ktl + 128 * r, 0.0, 1.0)
    identneg = (MASK_NEG * np.eye(128)).astype(np.float32)

    xTb = [np.ascontiguousarray(x[b].T) for b in range(B)]

    in_maps = []
    for c in range(NCORES):
        bi, hg = divmod(c, HPC)
        h0 = HPC * hg
        qwT_c = np.ascontiguousarray(qw[h0 * HD:(h0 + HPC) * HD, :].T)
        kwT_c = np.ascontiguousarray(kw[hg * HD:(hg + 1) * HD, :].T)
        vwT_c = np.ascontiguousarray(vw[hg * HD:(hg + 1) * HD, :].T)
        owT_c = ow[:, h0 * HD:(h0 + HPC) * HD].T.astype(bf).copy()
        # norm constants: s_i folds qg gain and 1/sqrt(HD) attention scale
        s = np.array([qg[h0 + i] / np.sqrt(HD) for i in range(HPC)] + [1.0],
                     np.float32)
        normo = np.broadcast_to(
            (1.0 / (HD * s * s))[None, :, None], (128, HPC + 1, 128)
        ).astype(np.float32).copy()
        normb = np.broadcast_to(
            (EPS / (s * s))[None, :], (128, HPC + 1)).astype(np.float32).copy()
        in_maps.append({
            "xT": xTb[bi], "qwT": qwT_c, "kwT": kwT_c, "vwT": vwT_c,
            "owT": owT_c, "csd": cossin, "csd2": sincos, "maskd": mask,
            "normod": normo, "normbd": normb,
            "onesd": np.ones((128, 1), np.float32),
            "identd": identneg,
        })
    return in_maps


def kernel(x, qw, kw, vw, ow, qg):
    global LAST_EXEC_NS
    x = np.ascontiguousarray(x, dtype=np.float32)
    qw = np.asarray(qw, dtype=np.float32)
    kw = np.asarray(kw, dtype=np.float32)
    vw = np.asarray(vw, dtype=np.float32)
    ow = np.asarray(ow, dtype=np.float32)
    qg = np.asarray(qg, dtype=np.float32)

    if "nc" not in _CACHED:
        _CACHED["nc"] = _build_program()
    nc = _CACHED["nc"]

    in_maps = _make_in_maps(x, qw, kw, vw, ow, qg)
    res = _run(nc, in_maps)
    LAST_EXEC_NS = res.exec_time_ns
    out = np.zeros((B, T, D), np.float64)
    for c in range(NCORES):
        bi = c // HPC
        out[bi] += res.results[c]["o"].astype(np.float64)
    return out.astype(np.float32)


# revision 20
# speedup vs baseline: 1.0560x; 1.0560x over previous
"""Bass/Tile kernel for nn_Attn_40424232189956 on 8 trn2 NeuronCores.

GQA attention block: q/k/v proj + rmsnorm + rope + causal attention + out proj.
B=2, T=2048, D=2048, NH=16, NKV=4, HD=128.

Sharding: 4 q-heads x 1 batch per core (core c: batch c//4, q heads
4*(c%4)..4*(c%4)+3, kv head c%4). Each (batch, kv head) pair is computed by
exactly one core -> no duplicated kv projection work. Each core emits a full
[T, D] partial of the output projection for its batch; host sums the 4
partials per batch.

Per-core kernel layout:
- Projections feat-major: psum [feat 128, tok 512], lhsT = W^T k-tiles,
  rhs = x^T k-tiles (x transposed + cast to bf16 on host). One batched DMA
  per 512-token chunk loads all 16 k-tiles.
- RMSNorm via ones-matmul partition reduction (value 1/(128*s_h^2) folds the
  qg gain and softmax 1/sqrt(HD) into the norm factor), sqrt bias eps/s_h^2.
- Rope in hd-major reading q halves straight from PSUM.
- Attention with TRANSPOSED scores sT[kt, qt]: softmax denominator via
  ones-column matmul (partition reduction on PE), p used directly as rhs of
  the pv matmul. exp() without max-subtraction (scores bounded by sqrt(HD)
  after rmsnorm). Heads processed in 2 passes of 2 (psum budget); the j-loop
  is software-pipelined: scores for step j issue before the sms/pv matmuls
  of step j-1, so the PE never waits on the exp() round trip.
- Causal masking: additive -30000 masks for the 4 diagonal block phases.
- Output written bf16 [T, D]; host sums partials in f32.
"""

import numpy as np

B, T, D = 2, 2048, 2048
NH, NKV = 16, 4
HD = 128
NCORES = 8
HPC = 4               # q heads per core
NKT = D // 128        # 16 contraction tiles for projections
CHUNK = 512
NCH = T // CHUNK      # 4 chunks
EPS = float(np.finfo(np.float32).eps)
MASK_NEG = -30000.0


def _rope_tables():
    # Matches reference.rotary_tables for T=2048 > tsl=1024 (NTK branch).
    hd = np.float32(HD)
    ar = (np.arange(0, HD, 2, dtype=np.float32) / hd).astype(np.float32)
    expo = np.power(np.float32(HD / (HD - 2.0)), ar, dtype=np.float32)
    inv = (np.float32(1.0)
           / (np.float32(10000.0)
              * np.power(np.float32(T / 1024.0), expo, dtype=np.float32)))
    f = np.outer(np.arange(T, dtype=np.float32), inv.astype(np.float32))
    return (np.cos(f).astype(np.float32).T.copy(),
            np.sin(f).astype(np.float32).T.copy())  # [64, T] hd-major


def _build_program():
    import concourse.bass as bass
    import concourse.mybir as mybir
    import concourse.tile as tile
    from concourse import bacc
    from concourse.masks import make_identity

    f32 = mybir.dt.float32
    f32r = mybir.dt.float32r
    bf16 = mybir.dt.bfloat16
    nc = bacc.Bacc("TRN2", target_bir_lowering=False)

    # lhsT (stationary) tensors are float32r: 4-byte weights self-load inside
    # the matmul, so tile_legalize emits no separate Ldweights instruction
    # (saves ~100ns of PE sequencer time per matmul). rhs (moving) tensors
    # are bf16: the PE row rate is keyed on the moving dtype.
    xT = nc.dram_tensor("xT", [D, T], f32r, kind="ExternalInput")
    qwT = nc.dram_tensor("qwT", [D, HPC * HD], f32r, kind="ExternalInput")
    kwT = nc.dram_tensor("kwT", [D, HD], f32r, kind="ExternalInput")
    vwT = nc.dram_tensor("vwT", [D, HD], f32r, kind="ExternalInput")
    owT = nc.dram_tensor("owT", [HPC * HD, D], bf16, kind="ExternalInput")
    csd = nc.dram_tensor("csd", [128, T], f32, kind="ExternalInput")
    csd2 = nc.dram_tensor("csd2", [128, T], f32, kind="ExternalInput")
    maskd = nc.dram_tensor("maskd", [128, 4, CHUNK], f32r, kind="ExternalInput")
    identd = nc.dram_tensor("identd", [128, 128], f32r, kind="ExternalInput")
    normod = nc.dram_tensor("normod", [128, HPC + 1, 128], f32r,
                            kind="ExternalInput")
    normbd = nc.dram_tensor("normbd", [128, HPC + 1], f32, kind="ExternalInput")
    onesd = nc.dram_tensor("onesd", [128, 1], f32r, kind="ExternalInput")
    outd = nc.dram_tensor("o", [T, D], bf16, kind="ExternalOutput")

    with tile.TileContext(nc) as tc:
        with (
            tc.tile_pool(name="wpool", bufs=1) as wpool,
            tc.tile_pool(name="xpool", bufs=2) as xpool,
            tc.tile_pool(name="big", bufs=1) as big,
            tc.tile_pool(name="ybp", bufs=2) as ybp,
            tc.tile_pool(name="ntmp", bufs=2) as ntmp,
            tc.tile_pool(name="ntmp1", bufs=2) as ntmp1,
            tc.tile_pool(name="atmp", bufs=2) as atmp,
            tc.tile_pool(name="ppool", bufs=4) as ppool,
            tc.tile_pool(name="opool", bufs=2) as opool,
            tc.tile_pool(name="ps", bufs=6, space="PSUM") as ps,
            tc.tile_pool(name="psv", bufs=2, space="PSUM") as psv,
        ):
            # ---- resident weights / tables ----
            # x and qw quarter-loads are interleaved on the SP queue in the
            # order the projection consumes them (ko-major); bulky tables
            # that are needed later (rope cos/sin, ow, mask) go out on the
            # DVE hardware-DGE queue so they never starve the x stream.
            NQ = NKT // 4
            xr = xT.rearrange("(ko p) m -> p ko m", p=128)
            qw_s = wpool.tile([128, NKT, HPC * HD], f32r)
            qwr = qwT.rearrange("(ko p) m -> p ko m", p=128)
            kw_s = wpool.tile([128, NKT, HD], f32r)
            vw_s = wpool.tile([128, NKT, HD], f32r)
            xc0 = xpool.tile([128, NQ, CHUNK], f32r, tag="xc",
                             name="xc_0_0")
            nc.sync.dma_start(xc0[:], xr[:, 0:NQ, 0:CHUNK])
            nc.sync.dma_start(qw_s[:, 0:4, :], qwr[:, 0:4, :])
            nc.sync.dma_start(kw_s[:], kwT.rearrange("(ko p) m -> p ko m", p=128))
            nc.sync.dma_start(vw_s[:], vwT.rearrange("(ko p) m -> p ko m", p=128))
            xc0q = [xc0]
            for qq in range(1, 4):
                xc0q.append(xpool.tile([128, NQ, CHUNK], f32r, tag="xc",
                                       name=f"xc_0_{qq}"))
                nc.sync.dma_start(xc0q[qq][:],
                                  xr[:, NQ * qq:NQ * (qq + 1), 0:CHUNK])
                nc.sync.dma_start(qw_s[:, 4 * qq:4 * qq + 4, :],
                                  qwr[:, 4 * qq:4 * qq + 4, :])
            normo_s = wpool.tile([128, HPC + 1, 128], f32r)
            nc.sync.dma_start(normo_s[:], normod[:])
            normb_s = wpool.tile([128, HPC + 1], f32)
            nc.sync.dma_start(normb_s[:], normbd[:])
            ones_col = wpool.tile([128, 1], f32r)
            nc.sync.dma_start(ones_col[:], onesd[:])
            # bulk tables on the Pool software-DGE queue (idle at startup)
            cs_s = wpool.tile([128, T], f32)  # rows 0:64 cos, 64:128 sin
            nc.gpsimd.dma_start(cs_s[:], csd[:])
            cs2_s = wpool.tile([128, T], f32)  # rows 0:64 sin, 64:128 cos
            nc.gpsimd.dma_start(cs2_s[:], csd2[:])
            mask_s = wpool.tile([128, 4, CHUNK], f32r)
            nc.gpsimd.dma_start(mask_s[:], maskd[:])
            identneg = wpool.tile([128, 128], f32r)
            nc.gpsimd.dma_start(identneg[:], identd[:])
            ow_s = wpool.tile([128, HPC, D], bf16)
            nc.gpsimd.dma_start(ow_s[:], owT.rearrange("(h p) n -> p h n", p=128))
            ident = wpool.tile([128, 128], f32)
            make_identity(nc, ident[:])

            qT = big.tile([128, HPC, T], bf16, tag="qT", name="qT")
            kT = big.tile([128, T], bf16, tag="kT", name="kT")
            vtok = big.tile([128, T], f32r, tag="vtok", name="vtok")

            sq_ = mybir.ActivationFunctionType.Square
            sqrt_ = mybir.ActivationFunctionType.Sqrt
            exp_ = mybir.ActivationFunctionType.Exp

            def norm_rope(pt, ni, dst, pos0):
                """pt: psum [128 feat, 512 tok]; ni: 0..3 q-head, 4 k;
                dst: sbuf [128, 512] slice; pos0: seq position of col 0."""
                sq = ntmp.tile([128, CHUNK], f32r, tag="sq")
                nc.scalar.activation(out=sq[:], in_=pt[:], func=sq_)
                # full-width copy: every rope operand then lives in SBUF, so
                # the projection PSUM bank frees after just two ACT reads
                qsb = ntmp.tile([128, CHUNK], f32, tag="qsb")
                nc.scalar.copy(out=qsb[:], in_=pt[:])
                nb = psv.tile([128, CHUNK], f32, tag="aux", name=f"nb_{ni}_{pos0}")
                nc.tensor.matmul(nb[:], normo_s[:, ni, :], sq[:],
                                 start=True, stop=True)
                rs = ntmp1.tile([64, CHUNK], f32, tag="rs")
                nc.scalar.activation(out=rs[:], in_=nb[0:64, :], func=sqrt_,
                                     bias=normb_s[0:64, ni:ni + 1], scale=1.0)
                rfac = ntmp1.tile([64, CHUNK], f32, tag="rf")
                nc.vector.reciprocal(rfac[:], rs[:])
                cs = cs_s[0:64, pos0:pos0 + CHUNK]       # cos @ base 0
                sn = cs_s[64:128, pos0:pos0 + CHUNK]     # sin @ base 64
                sn0 = cs2_s[0:64, pos0:pos0 + CHUNK]     # sin @ base 0
                cs64 = cs2_s[64:128, pos0:pos0 + CHUNK]  # cos @ base 64
                # three multiplies on the idle GPSIMD engine (sbuf-only
                # operands; each operand pair shares a partition window)
                t1 = ntmp1.tile([64, CHUNK], f32, tag="ta")
                t2 = ntmp1.tile([64, CHUNK], f32, tag="tb")
                nc.gpsimd.tensor_mul(t1[:], qsb[0:64, :], cs)
                nc.gpsimd.tensor_mul(t2[:], qsb[64:128, :], sn)
                nc.vector.tensor_add(t1[:], t1[:], t2[:])
                nc.vector.tensor_mul(dst[0:64, :], t1[:], rfac[:])
                t3 = ntmp1.tile([64, CHUNK], f32, tag="tb")
                t4 = ntmp1.tile([64, CHUNK], f32, tag="ta")
                nc.gpsimd.tensor_mul(t3[:], qsb[0:64, :], sn0)
                nc.vector.tensor_mul(t4[:], qsb[64:128, :], cs64)
                nc.vector.tensor_sub(t4[:], t4[:], t3[:])
                nc.vector.tensor_mul(dst[64:128, :], t4[:], rfac[:])

            def proj_chunk(ci):
                pos0 = ci * CHUNK
                if ci == 0:
                    xh = xc0q
                else:
                    xh = []
                    for qq in range(4):
                        xh.append(xpool.tile([128, NQ, CHUNK], f32r, tag="xc",
                                             name=f"xc_{ci}_{qq}"))
                        nc.sync.dma_start(
                            xh[qq][:],
                            xr[:, NQ * qq:NQ * (qq + 1), pos0:pos0 + CHUNK])
                pq = [ps.tile([128, CHUNK], f32, tag="b512",
                              name=f"pq{h}_{ci}") for h in range(HPC)]
                pk = ps.tile([128, CHUNK], f32, tag="b512", name=f"pk_{ci}")
                pv = ps.tile([128, CHUNK], f32, tag="b512", name=f"pv_{ci}")
                for ko in range(NKT):
                    rhs = xh[ko // NQ][:, ko % NQ, :]
                    st = (ko == 0)
                    sp = (ko == NKT - 1)
                    for h in range(HPC):
                        nc.tensor.matmul(pq[h][:],
                                         qw_s[:, ko, h * 128:(h + 1) * 128],
                                         rhs, start=st, stop=sp)
                    nc.tensor.matmul(pk[:], kw_s[:, ko, :], rhs,
                                     start=st, stop=sp)
                    nc.tensor.matmul(pv[:], vw_s[:, ko, :], rhs,
                                     start=st, stop=sp)
                # v first: psum [hd, tok] -> sbuf, PE-transpose to token-major
                vtmp = atmp.tile([128, CHUNK], f32, tag="vtmp",
                                 name=f"vtmp_{ci}")
                nc.scalar.copy(out=vtmp[:], in_=pv[:])
                for tb in range(4):
                    vps = psv.tile([128, CHUNK], f32, tag="aux",
                                   name=f"vps_{ci}_{tb}")[:, 0:128]
                    nc.tensor.transpose(
                        vps, vtmp[:, tb * 128:(tb + 1) * 128], ident[:])
                    dst0 = pos0 + tb * 128
                    nc.scalar.copy(out=vtok[:, dst0:dst0 + 128], in_=vps)
                # K norm/rope first: the next attention group's scores need it
                norm_rope(pk, HPC, kT[:, pos0:pos0 + CHUNK], pos0)
                for h in range(HPC):
                    norm_rope(pq[h], h, qT[:, h, pos0:pos0 + CHUNK], pos0)

            def attn_pass(g, hh):
                """Attention for query group g, heads hh (pair). Writes
                normalized per-head outputs into ybg[:, h, :]."""
                q0 = g * CHUNK
                kg = 4 * (g + 1)
                yts, sms = {}, {}
                for h in hh:
                    yts[h] = ps.tile([128, CHUNK], f32, tag="b512",
                                     name=f"yt_{g}_{h}")
                    sms[h] = psv.tile([128, CHUNK], f32, tag="aux",
                                      name=f"sm_{g}_{h}")[0:1, :]
                pend = None  # (j, {h: pj}) awaiting sms/pv issue
                for j in range(kg):
                    k0 = j * 128
                    st = {}
                    diag = j >= 4 * g
                    for h in hh:  # both scores share the kT lhsT
                        st[h] = ps.tile([128, CHUNK], f32, tag="b512",
                                        name=f"st_{g}_{h}_{j}")
                        nc.tensor.matmul(st[h][:], kT[:, k0:k0 + 128],
                                         qT[:, h, q0:q0 + CHUNK],
                                         start=True, stop=not diag)
                        if diag:
                            # causal mask on the PE: -30000*I @ mask01
                            # accumulates the additive mask into the score
                            # bank, keeping the DVE out of the exp chain
                            nc.tensor.matmul(st[h][:], identneg[:],
                                             mask_s[:, j - 4 * g, :],
                                             start=False, stop=True,
                                             skip_group_check=True)
                    if pend is not None:
                        pj_, j_ = pend
                        for h in hh:
                            nc.tensor.matmul(sms[h], ones_col[:], pj_[h][:],
                                             start=(j_ == 0), stop=False,
                                             skip_group_check=True)
                        for h in hh:
                            nc.tensor.matmul(yts[h][:],
                                             vtok[:, j_ * 128:j_ * 128 + 128],
                                             pj_[h][:],
                                             start=(j_ == 0), stop=False,
                                             skip_group_check=True)
                    pjs = {}
                    for h in hh:
                        pj = ppool.tile([128, CHUNK], f32r, tag="pj",
                                        name=f"pj_{g}_{h}_{j}")
                        nc.scalar.activation(out=pj[:], in_=st[h][:],
                                             func=exp_)
                        pjs[h] = pj
                    pend = (pjs, j)
                pj_, j_ = pend
                for h in hh:
                    nc.tensor.matmul(sms[h], ones_col[:], pj_[h][:],
                                     start=(j_ == 0), stop=True,
                                     skip_group_check=True)
                for h in hh:
                    nc.tensor.matmul(yts[h][:],
                                     vtok[:, j_ * 128:j_ * 128 + 128],
                                     pj_[h][:], start=(j_ == 0), stop=True,
                                     skip_group_check=True)
                for h in hh:
                    rrow = atmp.tile([1, CHUNK], f32, tag="rrow",
                                     name=f"rr_{g}_{h}")
                    nc.vector.reciprocal(rrow[:], sms[h])
                    rb = atmp.tile([128, CHUNK], f32, tag="rb",
                                   name=f"rb_{g}_{h}")
                    nc.gpsimd.partition_broadcast(rb[:], rrow[:])
                    nc.vector.tensor_mul(ybg_cur[0][:, h, :], yts[h][:], rb[:])

            def oproj_group(g):
                q0 = g * CHUNK
                ybg = ybg_cur[0]
                for tb in range(4):
                    row0 = q0 + tb * 128
                    ops = [ps.tile([128, CHUNK], f32, tag="b512",
                                   name=f"op_{g}_{tb}_{oc}")
                           for oc in range(4)]
                    for h in range(HPC):
                        lhsT = ybg[:, h, tb * 128:(tb + 1) * 128]
                        for oc in range(4):
                            nc.tensor.matmul(
                                ops[oc][:], lhsT,
                                ow_s[:, h, oc * 512:(oc + 1) * 512],
                                start=(h == 0), stop=(h == HPC - 1),
                                skip_group_check=True)
                    orow = opool.tile([128, D], bf16, tag="orow",
                                      name=f"or_{g}_{tb}")
                    for oc in range(4):
                        dst = orow[:, oc * 512:(oc + 1) * 512]
                        if oc % 2 == 0:
                            nc.vector.tensor_copy(out=dst, in_=ops[oc][:])
                        else:
                            nc.scalar.copy(out=dst, in_=ops[oc][:])
                    nc.sync.dma_start(outd[row0:row0 + 128, :], orow[:])

            ybg_cur = [None]
            # Projection of chunk c+2 is emitted before attention group c+1:
            # its norm/rope chain (DVE+GPSIMD) resolves while the PE runs the
            # attention matmuls, so group c+1 never waits on fresh q/k tiles.
            proj_chunk(0)
            proj_chunk(1)
            for c in range(NCH):
                ybg_cur[0] = ybp.tile([128, HPC, CHUNK], bf16, tag="ybg",
                                      name=f"ybg_{c}")
                attn_pass(c, (0, 1))
                attn_pass(c, (2, 3))
                oproj_group(c)
                if c + 2 < NCH:
                    proj_chunk(c + 2)

    nc.compile()
    return nc


_CACHED = {}
LAST_EXEC_NS = None


def _run(nc, in_maps, **kwargs):
    from concourse.bass_utils import run_bass_kernel_spmd
    return run_bass_kernel_spmd(nc, in_maps, core_ids=list(range(NCORES)),
                                **kwargs)


def _make_in_maps(x, qw, kw, vw, ow, qg):
    import ml_dtypes
    bf = ml_dtypes.bfloat16
    cosT, sinT = _rope_tables()
    cossin = np.concatenate([cosT, sinT], axis=0)   # [128, T] cos||sin
    sincos = np.concatenate([sinT, cosT], axis=0)   # [128, T] sin||cos

    ktl = np.arange(128, dtype=np.int64)[:, None]
    qtl = np.arange(CHUNK, dtype=np.int64)[None, :]
    mask = np.zeros((128, 4, CHUNK), np.float32)
    for r in range(4):
        mask[:, r, :] = np.where(qtl >= ktl + 128 * r, 0.0, 1.0)
    identneg = (MASK_NEG * np.eye(128)).astype(np.float32)

    xTb = [np.ascontiguousarray(x[b].T) for b in range(B)]

    in_maps = []
    for c in range(NCORES):
        bi, hg = divmod(c, HPC)
        h0 = HPC * hg
        qwT_c = np.ascontiguousarray(qw[h0 * HD:(h0 + HPC) * HD, :].T)
        kwT_c = np.ascontiguousarray(kw[hg * HD:(hg + 1) * HD, :].T)
        vwT_c = np.ascontiguousarray(vw[hg * HD:(hg + 1) * HD, :].T)
        owT_c = ow[:, h0 * HD:(h0 + HPC) * HD].T.astype(bf).copy()
        # norm constants: s_i folds qg gain and 1/sqrt(HD) attention scale
        s = np.array([qg[h0 + i] / np.sqrt(HD) for i in range(HPC)] + [1.0],
                     np.float32)
        normo = np.broadcast_to(
            (1.0 / (HD * s * s))[None, :, None], (128, HPC + 1, 128)
        ).astype(np.float32).copy()
        normb = np.broadcast_to(
            (EPS / (s * s))[None, :], (128, HPC + 1)).astype(np.float32).copy()
        in_maps.append({
            "xT": xTb[bi], "qwT": qwT_c, "kwT": kwT_c, "vwT": vwT_c,
            "owT": owT_c, "csd": cossin, "csd2": sincos, "maskd": mask,
            "normod": normo, "normbd": normb,
            "onesd": np.ones((128, 1), np.float32),
            "identd": identneg,
        })
    return in_maps


def kernel(x, qw, kw, vw, ow, qg):
    global LAST_EXEC_NS
    x = np.ascontiguousarray(x, dtype=np.float32)
    qw = np.asarray(qw, dtype=np.float32)
    kw = np.asarray(kw, dtype=np.float32)
    vw = np.asarray(vw, dtype=np.float32)
    ow = np.asarray(ow, dtype=np.float32)
    qg = np.asarray(qg, dtype=np.float32)

    if "nc" not in _CACHED:
        _CACHED["nc"] = _build_program()
    nc = _CACHED["nc"]

    in_maps = _make_in_maps(x, qw, kw, vw, ow, qg)
    res = _run(nc, in_maps)
    LAST_EXEC_NS = res.exec_time_ns
    out = np.zeros((B, T, D), np.float64)
    for c in range(NCORES):
        bi = c // HPC
        out[bi] += res.results[c]["o"].astype(np.float64)
    return out.astype(np.float32)
